# revision 20
# baseline (speedup 1.0000x reference)
"""Trainium2 Bass kernel for nn_CurvatureOnlyRegularizer (retrieval_knn).

Full inputs -> full output. Shards the 8192 points row-wise across 8 cores.

Per-core pipeline (1024 rows = 8 row-tiles of 128):
  1. S = e1 . e2^T - n_j via bf16 PE matmul with 2 augmented K-rows carrying
     -n_j (split hi/lo bf16). k-outer loop over 2-chunk PSUM groups reuses
     PE weights (LDWEIGHTS once per k per group).
  2. ACT evacuates PSUM as t1 = psum + (C0 - n_i) in f16 (t1 = C0 - d'^2,
     winners land in [150, 400] where f16 ulp <= 0.25).
  3. Per 1024-chunk: DVE max8 -> top-8 values, max_index -> chunk-local
     indices. 8 chunks -> 64 candidates. Candidates quantized (x12 + magic)
     and packed with idx/1024 in the f32 fraction; top-16-of-64 by
     max8/match_replace/max8 + max_index for the chunk id.
  4. Neighbor embeddings gathered via two dma_gather(transpose=True,
     prepare_only=True) + trigger_dma so the Pool engine only pays SWDGE
     desc-gen; transfers run on the DMA engines.
  5. PE gram (4 K-chunks per 128-col block); ACT evacuates to f16; hop-1
     DMAs extract diagonal 16x16 blocks into gstack; per-half 128-DMA fold
     converts to point-major rows.
  6. cos/sort/MSE phase runs in f16 (2x DVE): cos = raw*invd_l*invd_m,
     triu extract via ap_gather, 14 rounds of max8/match_replace sort,
     MSE vs host-reversed reference signatures accumulated on ACT.
Host sums the 8 per-core partial sums.
"""

import os
from contextlib import ExitStack

import ml_dtypes
import numpy as np

import concourse.bass as bass
import concourse.bass_isa as bass_isa
import concourse.mybir as mybir
import concourse.tile as tile
from concourse import bacc
from concourse.bass import ds, ts
from concourse.bass_utils import run_bass_kernel_spmd

N, D, K = 8192, 512, 15
NCORES = 8
SHARD = N // NCORES            # 1024
RT = SHARD // 128              # 8 row-tiles per core
NCH = N // 1024                # 8 column chunks of 1024
MAGIC = 12582912.0             # 1.5 * 2^23
C0 = 2200.0
QS = 12.0                      # candidate pack scale
PAD_CURV = -1.0
PAD_ANG = -4.0
NEG_F32 = -1.0e30
NEG_F16 = -60000.0
F32 = mybir.dt.float32
F16 = mybir.dt.float16
BF16 = mybir.dt.bfloat16
I16 = mybir.dt.int16
U32 = mybir.dt.uint32
AX = mybir.AxisListType
OP = mybir.AluOpType
AF = mybir.ActivationFunctionType

HALVES = [(0, 6), (6, 2)]


def build_nc(debug_out: bool = False):
    nc = bacc.Bacc("TRN2", target_bir_lowering=False, debug=False)

    rhsT_d = nc.dram_tensor("rhsT", [514, N], BF16, kind="ExternalInput")
    lhsT_d = nc.dram_tensor("lhsT", [514, SHARD], BF16, kind="ExternalInput")
    egat_d = nc.dram_tensor("egather", [N, D], BF16, kind="ExternalInput")
    bias_d = nc.dram_tensor("bias", [128, RT], F32, kind="ExternalInput")
    self_d = nc.dram_tensor("selfidx", [128, RT], F32, kind="ExternalInput")
    refc_d = nc.dram_tensor("refc", [SHARD, 16], F32, kind="ExternalInput")
    refa_d = nc.dram_tensor("refa", [SHARD, 112], F16, kind="ExternalInput")
    triu_d = nc.dram_tensor("triu", [128, 7], I16, kind="ExternalInput")
    part_d = nc.dram_tensor("partial", [1, 2], F32, kind="ExternalOutput")
    if debug_out:
        dbg_idx_d = nc.dram_tensor("dbg_idx", [128, 16], F32, kind="ExternalOutput")
        dbg_d2_d = nc.dram_tensor("dbg_d2", [128, 16], F32, kind="ExternalOutput")
        dbg_srtc_d = nc.dram_tensor("dbg_srtc", [128, 16], F32, kind="ExternalOutput")
        dbg_ang_d = nc.dram_tensor("dbg_ang", [128, 112], F16, kind="ExternalOutput")
        dbg_cand_d = nc.dram_tensor("dbg_cand", [128, 64], F32, kind="ExternalOutput")
        dbg_cv_d = nc.dram_tensor("dbg_cv", [128, 64], F32, kind="ExternalOutput")
        dbg_ci_d = nc.dram_tensor("dbg_ci", [128, 64], F32, kind="ExternalOutput")
        dbg_t1_d = nc.dram_tensor("dbg_t1", [128, 1024], F16, kind="ExternalOutput")
        dbg_cos_d = nc.dram_tensor("dbg_cos", [128, 256], F32, kind="ExternalOutput")
        dbg_ptr_d = nc.dram_tensor("dbg_ptr", [128, 256], F16, kind="ExternalOutput")
        dbg_iv_d = nc.dram_tensor("dbg_iv", [128, 16], F16, kind="ExternalOutput")

    r1024 = nc.gpsimd.to_reg(1024)
    gsems = [nc.alloc_semaphore(f"swdge_dma{i}") for i in range(8)]
    gsem_ctr = [0]
    # vt double-buffer reader guards: gram(t)'s last matmul on (buf, w) bumps
    # vtsem[buf*2+w]; trigger(t+2, w) waits for it before firing the DMA.
    vtsems = [nc.alloc_semaphore(f"vtsem{i}") for i in range(4)]
    rfill1 = nc.gpsimd.to_reg(1.0)

    with tile.TileContext(nc) as tc, ExitStack() as ctx:
        const = ctx.enter_context(tc.tile_pool(name="const", bufs=1))
        sel = ctx.enter_context(tc.tile_pool(name="sel", bufs=3))
        scr = ctx.enter_context(tc.tile_pool(name="scr", bufs=3))
        vbuf = ctx.enter_context(tc.tile_pool(name="vbuf", bufs=2))
        gbuf2 = ctx.enter_context(tc.tile_pool(name="gbuf2", bufs=2))
        psS = ctx.enter_context(tc.tile_pool(name="psS", bufs=1, space="PSUM"))
        psX = ctx.enter_context(tc.tile_pool(name="psX", bufs=1, space="PSUM"))
        psG = ctx.enter_context(tc.tile_pool(name="psG", bufs=1, space="PSUM"))

        # ---- constants / resident data ----
        rhs_sb = [const.tile([128, N], BF16, tag=f"rhs{c}", name=f"rhs{c}") for c in range(4)]
        rhs_aug = const.tile([2, N], BF16, tag="rhsaug")
        lhs_sb = [const.tile([128, SHARD], BF16, tag=f"lhs{c}", name=f"lhs{c}") for c in range(4)]
        lhs_aug = const.tile([2, SHARD], BF16, tag="lhsaug")
        bias_sb = const.tile([128, RT], F32, tag="bias")
        self_sb = const.tile([128, RT], F32, tag="self")
        triu_sb = const.tile([128, 7], I16, tag="triu")
        perm = const.tile([128, 128], F32, tag="perm")
        repmat = const.tile([16, 128], F32, tag="repmat")
        css = const.tile([128, 1], F32, tag="css")
        ass = const.tile([128, 1], F32, tag="ass")

        for c in range(4):
            for sl in range(4):
                nc.sync.dma_start(
                    rhs_sb[c][:, ts(sl, 2048)], rhsT_d.ap()[ts(c, 128), ts(sl, 2048)]
                )
            nc.sync.dma_start(lhs_sb[c][:], lhsT_d.ap()[ts(c, 128), :])
        nc.sync.dma_start(rhs_aug[:], rhsT_d.ap()[512:514, :])
        nc.sync.dma_start(lhs_aug[:], lhsT_d.ap()[512:514, :])
        nc.sync.dma_start(bias_sb[:], bias_d.ap()[:])
        nc.sync.dma_start(self_sb[:], self_d.ap()[:])
        nc.sync.dma_start(triu_sb[:], triu_d.ap()[:])
        # perm[k, 8u+v] = 1 iff k == 16v+u  (gram-position permutation)
        nc.gpsimd.memset(perm[:], 0.0)
        nc.gpsimd.affine_select(
            out=bass.AP(tensor=perm[:].tensor, offset=0,
                        ap=[perm[:].ap[0], [8, 16], [1, 8]]),
            in_=bass.AP(tensor=perm[:].tensor, offset=0,
                        ap=[perm[:].ap[0], [8, 16], [1, 8]]),
            compare_op=OP.not_equal,
            fill=rfill1, base=0,
            pattern=[[-1, 16], [-16, 8]],
            channel_multiplier=1,
        )
        # repmat[k, 16a+s] = 1 iff k == s  (partition-block replicator)
        nc.gpsimd.memset(repmat[:], 0.0)
        nc.gpsimd.affine_select(
            out=bass.AP(tensor=repmat[:].tensor, offset=0,
                        ap=[repmat[:].ap[0], [16, 8], [1, 16]]),
            in_=bass.AP(tensor=repmat[:].tensor, offset=0,
                        ap=[repmat[:].ap[0], [16, 8], [1, 16]]),
            compare_op=OP.not_equal,
            fill=rfill1, base=0,
            pattern=[[0, 8], [-1, 16]],
            channel_multiplier=1,
        )
        nc.vector.memset(css[:], 0.0)
        nc.vector.memset(ass[:], 0.0)

        def lhs_slice(k, t):
            if k < 4:
                return lhs_sb[k][:, ts(t, 128)]
            return lhs_aug[:, ts(t, 128)]

        def rhs_slice(k, j512):
            if k < 4:
                return rhs_sb[k][:, ts(j512, 512)]
            return rhs_aug[:, ts(j512, 512)]

        pending = []
        for half, (tbase, tbh) in enumerate(HALVES):
            gstack = gbuf2.tile([128, 16 * tbh * 16], F16, tag="gstack")
            invd_st = gbuf2.tile([128, 8, 16], F16, tag="invdst")
            for tt in range(tbh):
                t = tbase + tt
                # ================= phase A: matmul + evac + chunk top8 ====
                cand_v = sel.tile([128, 64], F16, tag="candv")
                cand_i = sel.tile([128, 64], U32, tag="candi")
                for g in range(4):
                    pss = [
                        psS.tile([128, 1024], F32, tag="psA", name="psA"),
                        psS.tile([128, 1024], F32, tag="psB", name="psB"),
                    ]
                    for k in range(5):
                        for q in range(2):
                            ch = 2 * g + q
                            for h in range(2):
                                nc.tensor.matmul(
                                    pss[q][:, ts(h, 512)],
                                    lhs_slice(k, t),
                                    rhs_slice(k, 2 * ch + h),
                                    start=(k == 0),
                                    stop=(k == 4),
                                )
                    for q in range(2):
                        ch = 2 * g + q
                        t1 = scr.tile([128, 1024], F16, tag=f"t1{q}")
                        nc.scalar.activation(
                            t1[:], pss[q][:], AF.Relu,
                            bias=bias_sb[:, t : t + 1], scale=1.0,
                        )
                        nc.vector.max(cand_v[:, ts(ch, 8)], t1[:])
                        nc.vector.max_index(
                            cand_i[:, ts(ch, 8)], cand_v[:, ts(ch, 8)], t1[:]
                        )
                        if debug_out and t == 0 and ch == 0:
                            nc.sync.dma_start(dbg_t1_d.ap()[:], t1[:])

                if pending:
                    pending.pop(0)()
                # ================= phase B: select top-16 + unpack ========
                candif = sel.tile([128, 64], F32, tag="candif")
                nc.vector.tensor_copy(candif[:], cand_i[:])
                nc.vector.tensor_scalar(
                    candif[:], candif[:], 1.0 / 1024.0, None, op0=OP.mult
                )
                candq = sel.tile([128, 64], F32, tag="candq")
                nc.vector.tensor_scalar(
                    candq[:], cand_v[:], QS, MAGIC, op0=OP.mult, op1=OP.add
                )
                cand3 = sel.tile([128, 64], F32, tag="cand3")
                nc.vector.scalar_tensor_tensor(
                    cand3[:], candq[:], -MAGIC, candif[:], op0=OP.add, op1=OP.add
                )
                v16 = sel.tile([128, 16], F32, tag="v16")
                pos = sel.tile([128, 16], U32, tag="pos")
                nc.vector.max(v16[:, 0:8], cand3[:])
                nc.vector.max_index(pos[:, 0:8], v16[:, 0:8], cand3[:])
                candz = sel.tile([128, 64], F32, tag="candz")
                nc.vector.match_replace(candz[:], v16[:, 0:8], cand3[:], NEG_F32)
                nc.vector.max(v16[:, 8:16], candz[:])
                nc.vector.max_index(pos[:, 8:16], v16[:, 8:16], candz[:])
                # unpack m (integer part) via magic round; frac = idx/1024
                s1 = sel.tile([128, 16], F32, tag="s1")
                nc.vector.tensor_scalar(
                    s1[:], v16[:], -0.49951171875, None, op0=OP.add
                )
                wv = sel.tile([128, 16], F32, tag="wv")
                nc.scalar.activation(wv[:], s1[:], AF.Copy, bias=MAGIC, scale=1.0)
                m16 = sel.tile([128, 16], F32, tag="m16")
                nc.vector.tensor_scalar(m16[:], wv[:], -MAGIC, None, op0=OP.add)
                frac = sel.tile([128, 16], F32, tag="frac")
                nc.vector.scalar_tensor_tensor(
                    frac[:], m16[:], -1.0, v16[:], op0=OP.mult, op1=OP.add
                )
                chunk_u = sel.tile([128, 16], U32, tag="chunku")
                nc.vector.tensor_scalar(
                    chunk_u[:], pos[:], 3, None, op0=OP.logical_shift_right
                )
                chunk_f = sel.tile([128, 16], F32, tag="chunkf")
                nc.vector.tensor_copy(chunk_f[:], chunk_u[:])
                gidx = sel.tile([128, 16], F32, tag="gidx")
                nc.vector.tensor_tensor(gidx[:], chunk_f[:], frac[:], op=OP.add)
                nc.vector.tensor_scalar(gidx[:], gidx[:], 1024.0, None, op0=OP.mult)
                # dp2 = C0 - m/QS (slot 0 = self, dropped)
                dp2 = sel.tile([128, 16], F32, tag="dp2")
                nc.vector.tensor_scalar(
                    dp2[:], m16[:], -1.0 / QS, C0, op0=OP.mult, op1=OP.add
                )
                # ================= phase C: curvature ======================
                d2re = sel.tile([128, 16], F32, tag="d2re")
                nc.vector.tensor_scalar_max(d2re[:, 0:15], dp2[:, 1:16], 1e-12)
                nc.vector.memset(d2re[:, 15:16], 1.0)
                dt_ = sel.tile([128, 16], F32, tag="dt")
                nc.scalar.sqrt(dt_[:], d2re[:])
                with nc.allow_low_precision(reason="invd f16 is plenty for cos"):
                    nc.vector.reciprocal(invd_st[:, tt, :], dt_[:])
                dsum = sel.tile([128, 1], F32, tag="dsum")
                nc.vector.reduce_sum(dsum[:], dt_[:, 0:15], axis=AX.X)
                dmean = sel.tile([128, 1], F32, tag="dmean")
                nc.vector.tensor_scalar(
                    dmean[:], dsum[:], 1.0 / 15.0, 1e-8, op0=OP.mult, op1=OP.add
                )
                ivm = sel.tile([128, 1], F32, tag="ivm")
                nc.vector.reciprocal(ivm[:], dmean[:])
                sig = sel.tile([128, 16], F32, tag="sig")
                nc.vector.tensor_scalar(
                    sig[:, 0:15], dt_[:, 0:15], ivm[:], None, op0=OP.mult
                )
                nc.vector.memset(sig[:, 15:16], PAD_CURV)
                srtc = sel.tile([128, 16], F32, tag="srtc")
                nc.vector.max(srtc[:, 0:8], sig[:])
                sigz = sel.tile([128, 16], F32, tag="sigz")
                nc.vector.match_replace(sigz[:], srtc[:, 0:8], sig[:], -2.0)
                nc.vector.max(srtc[:, 8:16], sigz[:])
                refc_t = scr.tile([128, 16], F32, tag="refct")
                nc.sync.dma_start(refc_t[:], refc_d.ap()[ts(t, 128), :])
                dcv = sel.tile([128, 16], F32, tag="dcv")
                nc.vector.tensor_tensor(
                    dcv[:], srtc[:], refc_t[:], op=OP.subtract
                )
                csq = sel.tile([128, 16], F32, tag="csq")
                css_t = sel.tile([128, 1], F32, tag="csst")
                nc.scalar.activation(csq[:], dcv[:], AF.Square, accum_out=css_t[:])
                nc.vector.tensor_tensor(css[:], css[:], css_t[:], op=OP.add)
                # ================= phase D: gather + gram ==================
                kif = sel.tile([128, 16], F32, tag="kif")
                nc.vector.tensor_copy(kif[:, 0:15], gidx[:, 1:16])
                nc.vector.tensor_copy(kif[:, 15:16], self_sb[:, t : t + 1])
                # idx16[16a+s, Q] = kif[phi(Q), s]: transpose (perm rhs),
                # then replicate the 16-row block via a constant matmul.
                pst1 = psX.tile([16, 128], F32, tag="pst1", name="pst1")
                nc.tensor.transpose(pst1[:], kif[:], perm[:])
                t1s = sel.tile([16, 128], F32, tag="t1s")
                nc.vector.tensor_copy(t1s[:], pst1[:])
                pst = psX.tile([128, 128], F32, tag="pstT", name="pst")
                nc.tensor.matmul(pst[:], repmat[:], t1s[:], start=True, stop=True)
                idx16 = sel.tile([128, 128], I16, tag="idx16")
                nc.vector.tensor_copy(idx16[:], pst[:])
                vts = [
                    vbuf.tile([128, 4, 1024], BF16, tag="vtA", name="vtA"),
                    vbuf.tile([128, 4, 1024], BF16, tag="vtB", name="vtB"),
                ]
                for w in range(2):
                    nc.gpsimd.dma_gather(
                        out_ap=vts[w][:],
                        in_ap=egat_d.ap()[:],
                        idxs_ap=idx16[:, ts(w, 64)],
                        num_idxs=1024,
                        num_idxs_reg=r1024,
                        elem_size=512,
                        transpose=True,
                        prepare_only=True,
                        sem=gsems[gsem_ctr[0] % 8],
                        single_packet=False,
                    )
                    gsem_ctr[0] += 1
                    trig = nc.gpsimd.trigger_dma(count=None)
                    if t >= 2:
                        trig.wait_op(
                            vtsems[(t % 2) * 2 + w], t // 2, "sem-ge"
                        )

                g0 = gsem_ctr[0] - 2

                def gram_phase(tt=tt, t=t, vts=vts, gstack=gstack, tbh=tbh,
                               g0=g0):
                    for w in range(2):
                        vt = vts[w]
                        gi = g0 + w
                        pg = psG.tile([128, 1024], F32, tag="pg", name="pg")
                        # standalone PE wait: gates LDWEIGHTS (which reads vt)
                        # as well as the matmuls on gather-DMA completion
                        nc.tensor.wait_ge(gsems[gi % 8], 16 * (gi // 8 + 1))
                        for g2h in range(8):
                            for c in range(4):
                                nc.tensor.matmul(
                                    pg[:, ts(g2h, 128)],
                                    vt[:, c, ts(g2h, 128)],
                                    vt[:, c, ts(g2h, 128)],
                                    start=(c == 0),
                                    stop=(c == 3),
                                )

                        gsk_t = scr.tile([128, 1024], F16, tag="gskt", name="gskt")
                        nc.scalar.activation(gsk_t[:], pg[:], AF.Copy)
                        nc.scalar.maybe_drain_then_inc(
                            (vtsems[(t % 2) * 2 + w], 1), fusable=True
                        )
                        # hop-1: gstack[16p+l, (g2*tbh + tt)*16 + m]
                        gfs = 16 * tbh * 16
                        for p in range(8):
                            h1_src = bass.AP(
                                tensor=gsk_t[:].tensor,
                                offset=gsk_t[:].offset + 16 * p * 1024 + 16 * p,
                                ap=[[1024, 16], [128, 8], [1, 16]],
                            )
                            h1_dst = bass.AP(
                                tensor=gstack[:].tensor,
                                offset=gstack[:].offset + 16 * p * gfs
                                + 16 * tbh * 8 * w + tt * 16,
                                ap=[[gfs, 16], [16 * tbh, 8], [1, 16]],
                            )
                            eng = nc.sync if (p + w) % 2 == 0 else nc.scalar
                            eng.dma_start(h1_dst, h1_src)
                pending.append(gram_phase)
                if debug_out and t == 0:
                    cvf = sel.tile([128, 64], F32, tag="cvf")
                    nc.vector.tensor_copy(cvf[:], cand_v[:])
                    nc.sync.dma_start(dbg_cv_d.ap()[:], cvf[:])
                    cif = sel.tile([128, 64], F32, tag="cif")
                    nc.vector.tensor_copy(cif[:], cand_i[:])
                    nc.sync.dma_start(dbg_ci_d.ap()[:], cif[:])
                    nc.sync.dma_start(dbg_idx_d.ap()[:], kif[:])
                    nc.sync.dma_start(dbg_d2_d.ap()[:], dp2[:])
                    nc.sync.dma_start(dbg_srtc_d.ap()[:], srtc[:])
                    nc.sync.dma_start(dbg_cand_d.ap()[:], cand3[:])

            while pending:
                pending.pop(0)()
            # ================= phase E: fold DMAs ======================
            ptR = gbuf2.tile([128, 16 * tbh * 16], F16, tag="ptR")
            FSg = 16 * tbh * 16
            FSp = FSg
            TBM = tbh * 16
            for p in range(8):
                for l in range(16):
                    src = bass.AP(
                        tensor=gstack[:].tensor,
                        offset=gstack[:].offset + (16 * p + l) * FSg,
                        ap=[[FSg, 1], [TBM, 16], [1, TBM]],
                    )
                    dst = bass.AP(
                        tensor=ptR[:].tensor,
                        offset=ptR[:].offset + 16 * p * FSp + l * TBM,
                        ap=[[FSp, 16], [1, TBM]],
                    )
                    eng = nc.sync if (p * 16 + l) % 2 == 0 else nc.scalar
                    eng.dma_start(dst, src)

            # ================= phase F: cos + sort + MSE ===============
            for tt in range(tbh):
                t = tbase + tt
                prt = ptR[:]
                p0 = prt.ap[0]
                base = prt.offset + tt * 16
                pr = bass.AP(tensor=prt.tensor, offset=base,
                             ap=[p0, [TBM, 16], [1, 16]])
                # raw = R - R[l,15] - R[15,m] + R[15,15]
                in_l15 = bass.AP(tensor=prt.tensor, offset=base + 15,
                                 ap=[p0, [TBM, 16], [0, 16]])
                in_r15 = bass.AP(tensor=prt.tensor, offset=base + 15 * TBM,
                                 ap=[p0, [0, 16], [1, 16]])
                ta = sel.tile([128, 256], F16, tag="ta")
                nc.vector.tensor_tensor(ta[:], pr, in_l15, op=OP.subtract)
                tb = sel.tile([128, 256], F16, tag="tb")
                nc.vector.tensor_tensor(tb[:], ta[:], in_r15, op=OP.subtract)
                # cos in f32 so the triu ap_gather moves 4-byte elements
                # (tb + R1515) * invd_l ; then * invd_m
                ivt = invd_st[:, tt, :]
                iv_l = bass.AP(
                    tensor=ivt.tensor, offset=ivt.offset,
                    ap=[ivt.ap[0], [1, 16], [0, 16]],
                )
                iv_m = bass.AP(
                    tensor=ivt.tensor, offset=ivt.offset,
                    ap=[ivt.ap[0], [0, 16], [1, 16]],
                )
                r1515 = bass.AP(
                    tensor=prt.tensor, offset=base + 15 * TBM + 15,
                    ap=[p0, [1, 1]],
                )
                tcc = sel.tile([128, 256], F16, tag="tc")
                nc.vector.scalar_tensor_tensor(
                    tcc[:], tb[:], r1515, iv_l, op0=OP.add, op1=OP.mult
                )
                cosv = sel.tile([128, 256], F32, tag="cosv")
                nc.vector.tensor_tensor(cosv[:], tcc[:], iv_m, op=OP.mult)
                angf = sel.tile([128, 112], F32, tag="angf")
                nc.gpsimd.ap_gather(
                    out_ap=angf[:].rearrange("p (a b) -> p a b", b=1),
                    in_ap=cosv[:].rearrange("p (a b) -> p a b", b=1),
                    idxs_ap=triu_sb[:],
                    channels=128,
                    num_elems=256,
                    d=1,
                    num_idxs=112,
                )
                if debug_out and t == 0:
                    nc.sync.dma_start(dbg_cos_d.ap()[:], cosv[:])
                    nc.sync.dma_start(dbg_ptr_d.ap()[:], pr)
                    nc.sync.dma_start(dbg_iv_d.ap()[:], ivt)
                angv = sel.tile([128, 112], F16, tag="angv")
                nc.vector.tensor_copy(angv[:, 0:105], angf[:, 0:105])
                nc.vector.memset(angv[:, 105:112], PAD_ANG)
                srta = sel.tile([128, 112], F16, tag="srta")
                work = angv
                for r in range(14):
                    nc.vector.max(srta[:, ts(r, 8)], work[:])
                    if r < 13:
                        nwork = sel.tile([128, 112], F16, tag=f"work{r % 2}")
                        nc.vector.match_replace(
                            nwork[:], srta[:, ts(r, 8)], work[:], NEG_F16
                        )
                        work = nwork
                refa_t = scr.tile([128, 112], F16, tag="refat")
                nc.sync.dma_start(refa_t[:], refa_d.ap()[ts(t, 128), :])
                dav = sel.tile([128, 112], F16, tag="dav")
                nc.vector.tensor_tensor(dav[:], srta[:], refa_t[:], op=OP.subtract)
                asq = sel.tile([128, 112], F32, tag="asq")
                ass_t = sel.tile([128, 1], F32, tag="asst")
                nc.scalar.activation(asq[:], dav[:], AF.Square, accum_out=ass_t[:])
                nc.vector.tensor_tensor(ass[:], ass[:], ass_t[:], op=OP.add)
                if debug_out and t == 0:
                    nc.sync.dma_start(dbg_ang_d.ap()[:], srta[:])

        # ---- final reduce + output ----
        cred = const.tile([128, 1], F32, tag="cred")
        ared = const.tile([128, 1], F32, tag="ared")
        nc.gpsimd.partition_all_reduce(
            cred[:], css[:], channels=128, reduce_op=bass_isa.ReduceOp.add
        )
        nc.gpsimd.partition_all_reduce(
            ared[:], ass[:], channels=128, reduce_op=bass_isa.ReduceOp.add
        )
        fin = const.tile([1, 2], F32, tag="fin")
        nc.vector.tensor_copy(fin[0:1, 0:1], cred[0:1, :])
        nc.vector.tensor_copy(fin[0:1, 1:2], ared[0:1, :])
        nc.sync.dma_start(part_d.ap()[:], fin[:])

    nc.compile()
    return nc


# =====================  host side  =====================

def _prep_inputs(embeddings, reference_curvature_sig, reference_angular_sig):
    emb32 = np.asarray(embeddings, dtype=np.float32)
    refc = np.asarray(reference_curvature_sig, dtype=np.float32)
    refa = np.asarray(reference_angular_sig, dtype=np.float32)

    e1_bf = (np.float32(np.sqrt(2.0)) * emb32).astype(ml_dtypes.bfloat16)
    e1 = e1_bf.astype(np.float32)
    e2_bf = (2.0 * e1).astype(ml_dtypes.bfloat16)       # exact x2
    n1 = np.sum(e1.astype(np.float64) * e1.astype(np.float64), axis=1).astype(
        np.float32
    )
    nnhi = n1.astype(ml_dtypes.bfloat16).astype(np.float32)
    nnlo = (n1 - nnhi).astype(ml_dtypes.bfloat16).astype(np.float32)

    rhsT = np.empty((514, N), dtype=ml_dtypes.bfloat16)
    rhsT[:512] = e2_bf.T
    rhsT[512] = (-nnhi).astype(ml_dtypes.bfloat16)
    rhsT[513] = (-nnlo).astype(ml_dtypes.bfloat16)

    lhsT_full = np.empty((514, N), dtype=ml_dtypes.bfloat16)
    lhsT_full[:512] = e1_bf.T
    lhsT_full[512:] = np.float32(1.0)

    tri = [l * 16 + m for l in range(15) for m in range(l + 1, 15)]
    tri += [255] * 7
    tri = np.array(tri, dtype=np.int16)                 # 112 entries
    triu = np.zeros((128, 7), dtype=np.int16)
    for p in range(128):
        for s in range(7):
            triu[p, s] = tri[s * 16 + (p & 15)]

    shared = dict(
        rhsT=rhsT, egather=e1_bf, triu=triu,
    )
    per_core = []
    for c in range(NCORES):
        lo = c * SHARD
        sl = slice(lo, lo + SHARD)
        bias = (C0 - n1[sl].astype(np.float64)).astype(np.float32)
        bias_t = bias.reshape(RT, 128).T.copy()         # [128, RT]
        selfidx = (np.arange(lo, lo + SHARD, dtype=np.float32)
                   .reshape(RT, 128).T.copy())
        refc_c = np.full((SHARD, 16), PAD_CURV, dtype=np.float32)
        refc_c[:, 0:15] = refc[sl, ::-1]
        refa_c = np.full((SHARD, 112), PAD_ANG, dtype=np.float16)
        refa_c[:, 0:105] = refa[sl, ::-1].astype(np.float16)
        per_core.append(dict(
            shared,
            lhsT=np.ascontiguousarray(lhsT_full[:, sl]),
            bias=bias_t,
            selfidx=selfidx,
            refc=refc_c,
            refa=refa_c,
        ))
    return per_core


_NC_CACHE = {}


def run_cores(inputs, debug_out=False, **run_kwargs):
    key = debug_out
    if key not in _NC_CACHE:
        _NC_CACHE[key] = build_nc(debug_out=debug_out)
    nc = _NC_CACHE[key]
    in_maps = _prep_inputs(**inputs)
    res = run_bass_kernel_spmd(
        nc, in_maps, core_ids=list(range(NCORES)), **run_kwargs
    )
    return res


def kernel(embeddings, reference_curvature_sig, reference_angular_sig):
    res = run_cores(dict(
        embeddings=embeddings,
        reference_curvature_sig=reference_curvature_sig,
        reference_angular_sig=reference_angular_sig,
    ))
    css = 0.0
    ass = 0.0
    for r in res.results:
        css += float(r["partial"][0, 0])
        ass += float(r["partial"][0, 1])
    curv_loss = css / (N * 15)
    ang_loss = ass / (N * 105)
    out = np.float32(0.3 * curv_loss + 0.7 * ang_loss)
    return np.asarray(out, dtype=np.float32)


# revision 22
# speedup vs baseline: 1.0390x; 1.0390x over previous
"""Trainium2 Bass kernel for nn_CurvatureOnlyRegularizer (retrieval_knn).

Full inputs -> full output. Shards the 8192 points row-wise across 8 cores.

Per-core pipeline (1024 rows = 8 row-tiles of 128):
  1. S = e1 . e2^T - n_j via bf16 PE matmul with 2 augmented K-rows carrying
     -n_j (split hi/lo bf16). k-outer loop over 2-chunk PSUM groups reuses
     PE weights (LDWEIGHTS once per k per group).
  2. ACT evacuates PSUM as t1 = psum + (C0 - n_i) in f16 (t1 = C0 - d'^2,
     winners land in [150, 400] where f16 ulp <= 0.25).
  3. Per 1024-chunk: DVE max8 -> top-8 values, max_index -> chunk-local
     indices. 8 chunks -> 64 candidates. Candidates quantized (x12 + magic)
     and packed with idx/1024 in the f32 fraction; top-16-of-64 by
     max8/match_replace/max8 + max_index for the chunk id.
  4. Neighbor embeddings gathered via two dma_gather(transpose=True,
     prepare_only=True) + trigger_dma so the Pool engine only pays SWDGE
     desc-gen; transfers run on the DMA engines.
  5. PE gram (4 K-chunks per 128-col block); ACT evacuates to f16; hop-1
     DMAs extract diagonal 16x16 blocks into gstack; per-half 128-DMA fold
     converts to point-major rows.
  6. cos/sort/MSE phase runs in f16 (2x DVE): cos = raw*invd_l*invd_m,
     triu extract via ap_gather, 14 rounds of max8/match_replace sort,
     MSE vs host-reversed reference signatures accumulated on ACT.
Host sums the 8 per-core partial sums.
"""

import os
from contextlib import ExitStack

import ml_dtypes
import numpy as np

import concourse.bass as bass
import concourse.bass_isa as bass_isa
import concourse.mybir as mybir
import concourse.tile as tile
from concourse import bacc
from concourse.bass import ds, ts
from concourse.bass_utils import run_bass_kernel_spmd

N, D, K = 8192, 512, 15
NCORES = 8
SHARD = N // NCORES            # 1024
RT = SHARD // 128              # 8 row-tiles per core
NCH = N // 1024                # 8 column chunks of 1024
MAGIC = 12582912.0             # 1.5 * 2^23
C0 = 2200.0
QS = 12.0                      # candidate pack scale
PAD_CURV = -1.0
PAD_ANG = -4.0
NEG_F32 = -1.0e30
NEG_F16 = -60000.0
F32 = mybir.dt.float32
F16 = mybir.dt.float16
BF16 = mybir.dt.bfloat16
I16 = mybir.dt.int16
U32 = mybir.dt.uint32
AX = mybir.AxisListType
OP = mybir.AluOpType
AF = mybir.ActivationFunctionType

HALVES = [(0, 6), (6, 2)]


def build_nc(debug_out: bool = False):
    nc = bacc.Bacc("TRN2", target_bir_lowering=False, debug=False)

    rhsT_d = nc.dram_tensor("rhsT", [514, N], BF16, kind="ExternalInput")
    lhsT_d = nc.dram_tensor("lhsT", [514, SHARD], BF16, kind="ExternalInput")
    egat_d = nc.dram_tensor("egather", [N, D], BF16, kind="ExternalInput")
    bias_d = nc.dram_tensor("bias", [128, RT], F32, kind="ExternalInput")
    self_d = nc.dram_tensor("selfidx", [128, RT], F32, kind="ExternalInput")
    refc_d = nc.dram_tensor("refc", [SHARD, 16], F32, kind="ExternalInput")
    refa_d = nc.dram_tensor("refa", [SHARD, 112], F16, kind="ExternalInput")
    triu_d = nc.dram_tensor("triu", [128, 7], I16, kind="ExternalInput")
    part_d = nc.dram_tensor("partial", [1, 2], F32, kind="ExternalOutput")
    if debug_out:
        dbg_idx_d = nc.dram_tensor("dbg_idx", [128, 16], F32, kind="ExternalOutput")
        dbg_d2_d = nc.dram_tensor("dbg_d2", [128, 16], F32, kind="ExternalOutput")
        dbg_srtc_d = nc.dram_tensor("dbg_srtc", [128, 16], F32, kind="ExternalOutput")
        dbg_ang_d = nc.dram_tensor("dbg_ang", [128, 112], F16, kind="ExternalOutput")
        dbg_cand_d = nc.dram_tensor("dbg_cand", [128, 64], F32, kind="ExternalOutput")
        dbg_cv_d = nc.dram_tensor("dbg_cv", [128, 64], F32, kind="ExternalOutput")
        dbg_ci_d = nc.dram_tensor("dbg_ci", [128, 64], F32, kind="ExternalOutput")
        dbg_t1_d = nc.dram_tensor("dbg_t1", [128, 1024], F16, kind="ExternalOutput")
        dbg_cos_d = nc.dram_tensor("dbg_cos", [128, 256], F32, kind="ExternalOutput")
        dbg_ptr_d = nc.dram_tensor("dbg_ptr", [128, 256], F16, kind="ExternalOutput")
        dbg_iv_d = nc.dram_tensor("dbg_iv", [128, 16], F16, kind="ExternalOutput")

    r1024 = nc.gpsimd.to_reg(1024)
    gsems = [nc.alloc_semaphore(f"swdge_dma{i}") for i in range(8)]
    gsem_ctr = [0]
    # vt triple-buffer reader guards: gram(t)'s evac drain on (buf, w) bumps
    # vtsem[buf*2+w]; trigger(t+3, w) waits for it before firing the DMA.
    vtsems = [nc.alloc_semaphore(f"vtsem{i}") for i in range(6)]
    rfill1 = nc.gpsimd.to_reg(1.0)

    with tile.TileContext(nc) as tc, ExitStack() as ctx:
        const = ctx.enter_context(tc.tile_pool(name="const", bufs=1))
        sel = ctx.enter_context(tc.tile_pool(name="sel", bufs=3))
        scr = ctx.enter_context(tc.tile_pool(name="scr", bufs=3))
        vbuf = ctx.enter_context(tc.tile_pool(name="vbuf", bufs=3))
        gbuf2 = ctx.enter_context(tc.tile_pool(name="gbuf2", bufs=2))
        psS = ctx.enter_context(tc.tile_pool(name="psS", bufs=1, space="PSUM"))
        psX = ctx.enter_context(tc.tile_pool(name="psX", bufs=1, space="PSUM"))
        psG = ctx.enter_context(tc.tile_pool(name="psG", bufs=1, space="PSUM"))

        # ---- constants / resident data ----
        rhs_sb = [const.tile([128, N], BF16, tag=f"rhs{c}", name=f"rhs{c}") for c in range(4)]
        rhs_aug = const.tile([2, N], BF16, tag="rhsaug")
        lhs_sb = [const.tile([128, SHARD], BF16, tag=f"lhs{c}", name=f"lhs{c}") for c in range(4)]
        lhs_aug = const.tile([2, SHARD], BF16, tag="lhsaug")
        bias_sb = const.tile([128, RT], F32, tag="bias")
        self_sb = const.tile([128, RT], F32, tag="self")
        triu_sb = const.tile([128, 7], I16, tag="triu")
        perm = const.tile([128, 128], F32, tag="perm")
        repmat = const.tile([16, 128], F32, tag="repmat")
        css = const.tile([128, 1], F32, tag="css")
        ass = const.tile([128, 1], F32, tag="ass")

        for c in range(4):
            for sl in range(4):
                nc.sync.dma_start(
                    rhs_sb[c][:, ts(sl, 2048)], rhsT_d.ap()[ts(c, 128), ts(sl, 2048)]
                )
            nc.sync.dma_start(lhs_sb[c][:], lhsT_d.ap()[ts(c, 128), :])
        nc.sync.dma_start(rhs_aug[:], rhsT_d.ap()[512:514, :])
        nc.sync.dma_start(lhs_aug[:], lhsT_d.ap()[512:514, :])
        nc.sync.dma_start(bias_sb[:], bias_d.ap()[:])
        nc.sync.dma_start(self_sb[:], self_d.ap()[:])
        nc.sync.dma_start(triu_sb[:], triu_d.ap()[:])
        # perm[k, 8u+v] = 1 iff k == 16v+u  (gram-position permutation)
        nc.gpsimd.memset(perm[:], 0.0)
        nc.gpsimd.affine_select(
            out=bass.AP(tensor=perm[:].tensor, offset=0,
                        ap=[perm[:].ap[0], [8, 16], [1, 8]]),
            in_=bass.AP(tensor=perm[:].tensor, offset=0,
                        ap=[perm[:].ap[0], [8, 16], [1, 8]]),
            compare_op=OP.not_equal,
            fill=rfill1, base=0,
            pattern=[[-1, 16], [-16, 8]],
            channel_multiplier=1,
        )
        # repmat[k, 16a+s] = 1 iff k == s  (partition-block replicator)
        nc.gpsimd.memset(repmat[:], 0.0)
        nc.gpsimd.affine_select(
            out=bass.AP(tensor=repmat[:].tensor, offset=0,
                        ap=[repmat[:].ap[0], [16, 8], [1, 16]]),
            in_=bass.AP(tensor=repmat[:].tensor, offset=0,
                        ap=[repmat[:].ap[0], [16, 8], [1, 16]]),
            compare_op=OP.not_equal,
            fill=rfill1, base=0,
            pattern=[[0, 8], [-1, 16]],
            channel_multiplier=1,
        )
        nc.vector.memset(css[:], 0.0)
        nc.vector.memset(ass[:], 0.0)

        def lhs_slice(k, t):
            if k < 4:
                return lhs_sb[k][:, ts(t, 128)]
            return lhs_aug[:, ts(t, 128)]

        def rhs_slice(k, j512):
            if k < 4:
                return rhs_sb[k][:, ts(j512, 512)]
            return rhs_aug[:, ts(j512, 512)]

        def make_fold_F(tbase, tbh, gstack, invd_st):
            # ================= phase E: fold DMAs ======================
            ptR = gbuf2.tile([128, 16 * tbh * 16], F16, tag="ptR", name="ptR")
            FSg = 16 * tbh * 16
            FSp = FSg
            TBM = tbh * 16
            FOLD_ENGS = [nc.sync, nc.scalar, nc.sync, nc.scalar]
            for p in range(8):
                for l in range(16):
                    src = bass.AP(
                        tensor=gstack[:].tensor,
                        offset=gstack[:].offset + (16 * p + l) * FSg,
                        ap=[[FSg, 1], [TBM, 16], [1, TBM]],
                    )
                    dst = bass.AP(
                        tensor=ptR[:].tensor,
                        offset=ptR[:].offset + 16 * p * FSp + l * TBM,
                        ap=[[FSp, 16], [1, TBM]],
                    )
                    FOLD_ENGS[(p * 16 + l) % 4].dma_start(dst, src)

            # ================= phase F: cos + sort + MSE ===============
            for tt in range(tbh):
                t = tbase + tt
                prt = ptR[:]
                p0 = prt.ap[0]
                base = prt.offset + tt * 16
                pr = bass.AP(tensor=prt.tensor, offset=base,
                             ap=[p0, [TBM, 16], [1, 16]])
                # raw = R - R[l,15] - R[15,m] + R[15,15]
                in_l15 = bass.AP(tensor=prt.tensor, offset=base + 15,
                                 ap=[p0, [TBM, 16], [0, 16]])
                in_r15 = bass.AP(tensor=prt.tensor, offset=base + 15 * TBM,
                                 ap=[p0, [0, 16], [1, 16]])
                ta = sel.tile([128, 256], F16, tag="ta", name="ta")
                nc.vector.tensor_tensor(ta[:], pr, in_l15, op=OP.subtract)
                tb = sel.tile([128, 256], F16, tag="tb", name="tb")
                nc.vector.tensor_tensor(tb[:], ta[:], in_r15, op=OP.subtract)
                # cos in f32 so the triu ap_gather moves 4-byte elements
                ivt = invd_st[:, tt, :]
                iv_l = bass.AP(
                    tensor=ivt.tensor, offset=ivt.offset,
                    ap=[ivt.ap[0], [1, 16], [0, 16]],
                )
                iv_m = bass.AP(
                    tensor=ivt.tensor, offset=ivt.offset,
                    ap=[ivt.ap[0], [0, 16], [1, 16]],
                )
                r1515 = bass.AP(
                    tensor=prt.tensor, offset=base + 15 * TBM + 15,
                    ap=[p0, [1, 1]],
                )
                tcc = sel.tile([128, 256], F16, tag="tc", name="tcc")
                nc.vector.scalar_tensor_tensor(
                    tcc[:], tb[:], r1515, iv_l, op0=OP.add, op1=OP.mult
                )
                cosv = sel.tile([128, 256], F32, tag="cosv", name="cosv")
                nc.vector.tensor_tensor(cosv[:], tcc[:], iv_m, op=OP.mult)
                angf = sel.tile([128, 112], F32, tag="angf", name="angf")
                nc.gpsimd.ap_gather(
                    out_ap=angf[:].rearrange("p (a b) -> p a b", b=1),
                    in_ap=cosv[:].rearrange("p (a b) -> p a b", b=1),
                    idxs_ap=triu_sb[:],
                    channels=128,
                    num_elems=256,
                    d=1,
                    num_idxs=112,
                )
                if debug_out and t == 0:
                    nc.sync.dma_start(dbg_cos_d.ap()[:], cosv[:])
                    nc.sync.dma_start(dbg_ptr_d.ap()[:], pr)
                    nc.sync.dma_start(dbg_iv_d.ap()[:], ivt)
                angv = sel.tile([128, 112], F16, tag="angv", name="angv")
                nc.vector.tensor_copy(angv[:, 0:105], angf[:, 0:105])
                nc.vector.memset(angv[:, 105:112], PAD_ANG)
                srta = sel.tile([128, 112], F16, tag="srta", name="srta")
                work = angv
                for r in range(14):
                    nc.vector.max(srta[:, ts(r, 8)], work[:])
                    if r < 13:
                        nwork = sel.tile([128, 112], F16, tag=f"work{r % 2}",
                                         name="nwork")
                        nc.vector.match_replace(
                            nwork[:], srta[:, ts(r, 8)], work[:], NEG_F16
                        )
                        work = nwork
                refa_t = scr.tile([128, 112], F16, tag="refat", name="refa_t")
                nc.sync.dma_start(refa_t[:], refa_d.ap()[ts(t, 128), :])
                dav = sel.tile([128, 112], F16, tag="dav", name="dav")
                nc.vector.tensor_tensor(dav[:], srta[:], refa_t[:], op=OP.subtract)
                asq = sel.tile([128, 112], F32, tag="asq", name="asq")
                ass_t = sel.tile([128, 1], F32, tag="asst", name="ass_t")
                nc.scalar.activation(asq[:], dav[:], AF.Square, accum_out=ass_t[:])
                nc.vector.tensor_tensor(ass[:], ass[:], ass_t[:], op=OP.add)
                if debug_out and t == 0:
                    nc.sync.dma_start(dbg_ang_d.ap()[:], srta[:])

        pending = []
        half_tiles = {}
        H0 = HALVES[0][1]
        for t in range(RT):
            half = 0 if t < H0 else 1
            tbase, tbh = HALVES[half]
            tt = t - tbase
            if tt == 0:
                gstack = gbuf2.tile([128, 16 * tbh * 16], F16, tag="gstack",
                                    name="gstack")
                invd_st = gbuf2.tile([128, 8, 16], F16, tag="invdst",
                                     name="invd_st")
                half_tiles[half] = (gstack, invd_st)
            else:
                gstack, invd_st = half_tiles[half]
            if True:
                # ================= phase A: matmul + evac + chunk top8 ====
                cand_v = sel.tile([128, 64], F16, tag="candv")
                cand_i = sel.tile([128, 64], U32, tag="candi")
                for g in range(4):
                    pss = [
                        psS.tile([128, 1024], F32, tag="psA", name="psA"),
                        psS.tile([128, 1024], F32, tag="psB", name="psB"),
                    ]
                    for k in range(5):
                        for q in range(2):
                            ch = 2 * g + q
                            for h in range(2):
                                nc.tensor.matmul(
                                    pss[q][:, ts(h, 512)],
                                    lhs_slice(k, t),
                                    rhs_slice(k, 2 * ch + h),
                                    start=(k == 0),
                                    stop=(k == 4),
                                )
                    for q in range(2):
                        ch = 2 * g + q
                        t1 = scr.tile([128, 1024], F16, tag=f"t1{q}")
                        nc.scalar.activation(
                            t1[:], pss[q][:], AF.Relu,
                            bias=bias_sb[:, t : t + 1], scale=1.0,
                        )
                        nc.vector.max(cand_v[:, ts(ch, 8)], t1[:])
                        nc.vector.max_index(
                            cand_i[:, ts(ch, 8)], cand_v[:, ts(ch, 8)], t1[:]
                        )
                        if debug_out and t == 0 and ch == 0:
                            nc.sync.dma_start(dbg_t1_d.ap()[:], t1[:])

                if len(pending) >= 2:
                    pending.pop(0)()
                # ================= phase B: select top-16 + unpack ========
                candif = sel.tile([128, 64], F32, tag="candif")
                nc.vector.tensor_copy(candif[:], cand_i[:])
                nc.vector.tensor_scalar(
                    candif[:], candif[:], 1.0 / 1024.0, None, op0=OP.mult
                )
                candq = sel.tile([128, 64], F32, tag="candq")
                nc.vector.tensor_scalar(
                    candq[:], cand_v[:], QS, MAGIC, op0=OP.mult, op1=OP.add
                )
                cand3 = sel.tile([128, 64], F32, tag="cand3")
                nc.vector.scalar_tensor_tensor(
                    cand3[:], candq[:], -MAGIC, candif[:], op0=OP.add, op1=OP.add
                )
                v16 = sel.tile([128, 16], F32, tag="v16")
                pos = sel.tile([128, 16], U32, tag="pos")
                nc.vector.max(v16[:, 0:8], cand3[:])
                nc.vector.max_index(pos[:, 0:8], v16[:, 0:8], cand3[:])
                candz = sel.tile([128, 64], F32, tag="candz")
                nc.vector.match_replace(candz[:], v16[:, 0:8], cand3[:], NEG_F32)
                nc.vector.max(v16[:, 8:16], candz[:])
                nc.vector.max_index(pos[:, 8:16], v16[:, 8:16], candz[:])
                # unpack m (integer part) via magic round; frac = idx/1024
                s1 = sel.tile([128, 16], F32, tag="s1")
                nc.vector.tensor_scalar(
                    s1[:], v16[:], -0.49951171875, None, op0=OP.add
                )
                wv = sel.tile([128, 16], F32, tag="wv")
                nc.scalar.activation(wv[:], s1[:], AF.Copy, bias=MAGIC, scale=1.0)
                m16 = sel.tile([128, 16], F32, tag="m16")
                nc.vector.tensor_scalar(m16[:], wv[:], -MAGIC, None, op0=OP.add)
                frac = sel.tile([128, 16], F32, tag="frac")
                nc.vector.scalar_tensor_tensor(
                    frac[:], m16[:], -1.0, v16[:], op0=OP.mult, op1=OP.add
                )
                chunk_u = sel.tile([128, 16], U32, tag="chunku")
                nc.vector.tensor_scalar(
                    chunk_u[:], pos[:], 3, None, op0=OP.logical_shift_right
                )
                chunk_f = sel.tile([128, 16], F32, tag="chunkf")
                nc.vector.tensor_copy(chunk_f[:], chunk_u[:])
                gidx = sel.tile([128, 16], F32, tag="gidx")
                nc.vector.tensor_tensor(gidx[:], chunk_f[:], frac[:], op=OP.add)
                nc.vector.tensor_scalar(gidx[:], gidx[:], 1024.0, None, op0=OP.mult)
                # dp2 = C0 - m/QS (slot 0 = self, dropped)
                dp2 = sel.tile([128, 16], F32, tag="dp2")
                nc.vector.tensor_scalar(
                    dp2[:], m16[:], -1.0 / QS, C0, op0=OP.mult, op1=OP.add
                )
                # ================= phase C: curvature ======================
                d2re = sel.tile([128, 16], F32, tag="d2re")
                nc.vector.tensor_scalar_max(d2re[:, 0:15], dp2[:, 1:16], 1e-12)
                nc.vector.memset(d2re[:, 15:16], 1.0)
                dt_ = sel.tile([128, 16], F32, tag="dt")
                nc.scalar.sqrt(dt_[:], d2re[:])
                with nc.allow_low_precision(reason="invd f16 is plenty for cos"):
                    nc.vector.reciprocal(invd_st[:, tt, :], dt_[:])
                dsum = sel.tile([128, 1], F32, tag="dsum")
                nc.vector.reduce_sum(dsum[:], dt_[:, 0:15], axis=AX.X)
                dmean = sel.tile([128, 1], F32, tag="dmean")
                nc.vector.tensor_scalar(
                    dmean[:], dsum[:], 1.0 / 15.0, 1e-8, op0=OP.mult, op1=OP.add
                )
                ivm = sel.tile([128, 1], F32, tag="ivm")
                nc.vector.reciprocal(ivm[:], dmean[:])
                sig = sel.tile([128, 16], F32, tag="sig")
                nc.vector.tensor_scalar(
                    sig[:, 0:15], dt_[:, 0:15], ivm[:], None, op0=OP.mult
                )
                nc.vector.memset(sig[:, 15:16], PAD_CURV)
                srtc = sel.tile([128, 16], F32, tag="srtc")
                nc.vector.max(srtc[:, 0:8], sig[:])
                sigz = sel.tile([128, 16], F32, tag="sigz")
                nc.vector.match_replace(sigz[:], srtc[:, 0:8], sig[:], -2.0)
                nc.vector.max(srtc[:, 8:16], sigz[:])
                refc_t = scr.tile([128, 16], F32, tag="refct")
                nc.sync.dma_start(refc_t[:], refc_d.ap()[ts(t, 128), :])
                dcv = sel.tile([128, 16], F32, tag="dcv")
                nc.vector.tensor_tensor(
                    dcv[:], srtc[:], refc_t[:], op=OP.subtract
                )
                csq = sel.tile([128, 16], F32, tag="csq")
                css_t = sel.tile([128, 1], F32, tag="csst")
                nc.scalar.activation(csq[:], dcv[:], AF.Square, accum_out=css_t[:])
                nc.vector.tensor_tensor(css[:], css[:], css_t[:], op=OP.add)
                # ================= phase D: gather + gram ==================
                kif = sel.tile([128, 16], F32, tag="kif")
                nc.vector.tensor_copy(kif[:, 0:15], gidx[:, 1:16])
                nc.vector.tensor_copy(kif[:, 15:16], self_sb[:, t : t + 1])
                # idx16[16a+s, Q] = kif[phi(Q), s]: transpose (perm rhs),
                # then replicate the 16-row block via a constant matmul.
                pst1 = psX.tile([16, 128], F32, tag="pst1", name="pst1")
                nc.tensor.transpose(pst1[:], kif[:], perm[:])
                t1s = sel.tile([16, 128], F32, tag="t1s")
                nc.vector.tensor_copy(t1s[:], pst1[:])
                pst = psX.tile([128, 128], F32, tag="pstT", name="pst")
                nc.tensor.matmul(pst[:], repmat[:], t1s[:], start=True, stop=True)
                idx16 = sel.tile([128, 128], I16, tag="idx16")
                nc.vector.tensor_copy(idx16[:], pst[:])
                vts = [
                    vbuf.tile([128, 4, 1024], BF16, tag="vtA", name="vtA"),
                    vbuf.tile([128, 4, 1024], BF16, tag="vtB", name="vtB"),
                ]
                for w in range(2):
                    nc.gpsimd.dma_gather(
                        out_ap=vts[w][:],
                        in_ap=egat_d.ap()[:],
                        idxs_ap=idx16[:, ts(w, 64)],
                        num_idxs=1024,
                        num_idxs_reg=r1024,
                        elem_size=512,
                        transpose=True,
                        prepare_only=True,
                        sem=gsems[gsem_ctr[0] % 8],
                        single_packet=False,
                    )
                    gsem_ctr[0] += 1
                    trig = nc.gpsimd.trigger_dma(count=None)
                    if t >= 3:
                        trig.wait_op(
                            vtsems[(t % 3) * 2 + w], t // 3, "sem-ge"
                        )

                g0 = gsem_ctr[0] - 2

                def gram_phase(tt=tt, t=t, vts=vts, gstack=gstack, tbh=tbh,
                               g0=g0):
                    for w in range(2):
                        vt = vts[w]
                        gi = g0 + w
                        pg = psG.tile([128, 1024], F32, tag="pg", name="pg")
                        # standalone PE wait: gates LDWEIGHTS (which reads vt)
                        # as well as the matmuls on gather-DMA completion
                        nc.tensor.wait_ge(gsems[gi % 8], 16 * (gi // 8 + 1))
                        for g2h in range(8):
                            for c in range(4):
                                nc.tensor.matmul(
                                    pg[:, ts(g2h, 128)],
                                    vt[:, c, ts(g2h, 128)],
                                    vt[:, c, ts(g2h, 128)],
                                    start=(c == 0),
                                    stop=(c == 3),
                                )

                        gsk_t = scr.tile([128, 1024], F16, tag="gskt", name="gskt")
                        nc.scalar.activation(gsk_t[:], pg[:], AF.Copy)
                        nc.scalar.maybe_drain_then_inc(
                            (vtsems[(t % 3) * 2 + w], 1), fusable=True
                        )
                        # hop-1: gstack[16p+l, (g2*tbh + tt)*16 + m]
                        gfs = 16 * tbh * 16
                        for p in range(8):
                            h1_src = bass.AP(
                                tensor=gsk_t[:].tensor,
                                offset=gsk_t[:].offset + 16 * p * 1024 + 16 * p,
                                ap=[[1024, 16], [128, 8], [1, 16]],
                            )
                            h1_dst = bass.AP(
                                tensor=gstack[:].tensor,
                                offset=gstack[:].offset + 16 * p * gfs
                                + 16 * tbh * 8 * w + tt * 16,
                                ap=[[gfs, 16], [16 * tbh, 8], [1, 16]],
                            )
                            eng = nc.sync if (p + w) % 2 == 0 else nc.scalar
                            eng.dma_start(h1_dst, h1_src)
                pending.append(gram_phase)
                if debug_out and t == 0:
                    cvf = sel.tile([128, 64], F32, tag="cvf")
                    nc.vector.tensor_copy(cvf[:], cand_v[:])
                    nc.sync.dma_start(dbg_cv_d.ap()[:], cvf[:])
                    cif = sel.tile([128, 64], F32, tag="cif")
                    nc.vector.tensor_copy(cif[:], cand_i[:])
                    nc.sync.dma_start(dbg_ci_d.ap()[:], cif[:])
                    nc.sync.dma_start(dbg_idx_d.ap()[:], kif[:])
                    nc.sync.dma_start(dbg_d2_d.ap()[:], dp2[:])
                    nc.sync.dma_start(dbg_srtc_d.ap()[:], srtc[:])
                    nc.sync.dma_start(dbg_cand_d.ap()[:], cand3[:])

            # after phase D of the last row-tile: half-0 fold + F
            if t == RT - 1:
                while len(pending) > 2:
                    pending.pop(0)()
                g0s, g0i = half_tiles[0]
                make_fold_F(HALVES[0][0], HALVES[0][1], g0s, g0i)

        while pending:
            pending.pop(0)()
        g1s, g1i = half_tiles[1]
        make_fold_F(HALVES[1][0], HALVES[1][1], g1s, g1i)

        # ---- final reduce + output ----
        cred = const.tile([128, 1], F32, tag="cred")
        ared = const.tile([128, 1], F32, tag="ared")
        nc.gpsimd.partition_all_reduce(
            cred[:], css[:], channels=128, reduce_op=bass_isa.ReduceOp.add
        )
        nc.gpsimd.partition_all_reduce(
            ared[:], ass[:], channels=128, reduce_op=bass_isa.ReduceOp.add
        )
        fin = const.tile([1, 2], F32, tag="fin")
        nc.vector.tensor_copy(fin[0:1, 0:1], cred[0:1, :])
        nc.vector.tensor_copy(fin[0:1, 1:2], ared[0:1, :])
        nc.sync.dma_start(part_d.ap()[:], fin[:])

    nc.compile()
    return nc


# =====================  host side  =====================

def _prep_inputs(embeddings, reference_curvature_sig, reference_angular_sig):
    emb32 = np.asarray(embeddings, dtype=np.float32)
    refc = np.asarray(reference_curvature_sig, dtype=np.float32)
    refa = np.asarray(reference_angular_sig, dtype=np.float32)

    e1_bf = (np.float32(np.sqrt(2.0)) * emb32).astype(ml_dtypes.bfloat16)
    e1 = e1_bf.astype(np.float32)
    e2_bf = (2.0 * e1).astype(ml_dtypes.bfloat16)       # exact x2
    n1 = np.sum(e1.astype(np.float64) * e1.astype(np.float64), axis=1).astype(
        np.float32
    )
    nnhi = n1.astype(ml_dtypes.bfloat16).astype(np.float32)
    nnlo = (n1 - nnhi).astype(ml_dtypes.bfloat16).astype(np.float32)

    rhsT = np.empty((514, N), dtype=ml_dtypes.bfloat16)
    rhsT[:512] = e2_bf.T
    rhsT[512] = (-nnhi).astype(ml_dtypes.bfloat16)
    rhsT[513] = (-nnlo).astype(ml_dtypes.bfloat16)

    lhsT_full = np.empty((514, N), dtype=ml_dtypes.bfloat16)
    lhsT_full[:512] = e1_bf.T
    lhsT_full[512:] = np.float32(1.0)

    tri = [l * 16 + m for l in range(15) for m in range(l + 1, 15)]
    tri += [255] * 7
    tri = np.array(tri, dtype=np.int16)                 # 112 entries
    triu = np.zeros((128, 7), dtype=np.int16)
    for p in range(128):
        for s in range(7):
            triu[p, s] = tri[s * 16 + (p & 15)]

    shared = dict(
        rhsT=rhsT, egather=e1_bf, triu=triu,
    )
    per_core = []
    for c in range(NCORES):
        lo = c * SHARD
        sl = slice(lo, lo + SHARD)
        bias = (C0 - n1[sl].astype(np.float64)).astype(np.float32)
        bias_t = bias.reshape(RT, 128).T.copy()         # [128, RT]
        selfidx = (np.arange(lo, lo + SHARD, dtype=np.float32)
                   .reshape(RT, 128).T.copy())
        refc_c = np.full((SHARD, 16), PAD_CURV, dtype=np.float32)
        refc_c[:, 0:15] = refc[sl, ::-1]
        refa_c = np.full((SHARD, 112), PAD_ANG, dtype=np.float16)
        refa_c[:, 0:105] = refa[sl, ::-1].astype(np.float16)
        per_core.append(dict(
            shared,
            lhsT=np.ascontiguousarray(lhsT_full[:, sl]),
            bias=bias_t,
            selfidx=selfidx,
            refc=refc_c,
            refa=refa_c,
        ))
    return per_core


_NC_CACHE = {}


def run_cores(inputs, debug_out=False, **run_kwargs):
    key = debug_out
    if key not in _NC_CACHE:
        _NC_CACHE[key] = build_nc(debug_out=debug_out)
    nc = _NC_CACHE[key]
    in_maps = _prep_inputs(**inputs)
    res = run_bass_kernel_spmd(
        nc, in_maps, core_ids=list(range(NCORES)), **run_kwargs
    )
    return res


def kernel(embeddings, reference_curvature_sig, reference_angular_sig):
    res = run_cores(dict(
        embeddings=embeddings,
        reference_curvature_sig=reference_curvature_sig,
        reference_angular_sig=reference_angular_sig,
    ))
    css = 0.0
    ass = 0.0
    for r in res.results:
        css += float(r["partial"][0, 0])
        ass += float(r["partial"][0, 1])
    curv_loss = css / (N * 15)
    ang_loss = ass / (N * 105)
    out = np.float32(0.3 * curv_loss + 0.7 * ang_loss)
    return np.asarray(out, dtype=np.float32)
